# revision 5
# baseline (speedup 1.0000x reference)
"""Trainium2 Bass kernel for the CLRU transition model.

Data-parallel over batch: 8 cores x 256 batch each. T=128 sequential steps.
Activations live transposed in SBUF: [feature-partitions, batch-free].

Per step (per core), with zT [64,256] state and hzT [64,256]:
  h1 = relu((x@W1)*s1 + t1)        x = [z; u; dt] (K=81)
  h2 = relu((h1@W2)*s2 + t2)
  ps3 = [Wnu | W3] contraction     -> nu-pre rows 0:64, hz-pre rows 64:128
  Lam = exp(-exp(min(nu + bnu, 2)))
  z1add/y_d = WBDp.T @ G           G = udt (x) hz outer product  [1024+16, 256]
  y_c       = WCp.T  @ G2          G2 = z1  (x) hz outer product [4096+64, 256]
  z1 = Lam*z + z1add ; y = y_d + y_c

Outer products are built chunkwise (128 rows): a selector matmul broadcasts
pairs of z1/udt rows across partitions into PSUM, then DVE multiplies by
HZ2 = [hz; hz].  Bias terms bB/bD/bC are folded in as extra contraction
chunks whose rhs are udtT / z1T themselves.
"""

import numpy as np

LATENT, UDIM, NY = 64, 16, 50
H1, H2 = 128, 128
B, T = 2048, 128
DIN = LATENT + UDIM + 1
EPS = 1e-5
NU_MAX = 2.0
NCORES = 8
BS = B // NCORES          # 256 batch per core
NKC = LATENT // 2         # 32 G2 chunks (2 n-rows x 64 k each)
NUC = UDIM // 2           # 8 G chunks

MM_DT = "float32"         # matmul dtype ("float32" exact, "float32r" fast/TF32-like)
ELEM_DT = "float32"       # dtype of outer-product chunks fed to big matmuls


def _host_prep(zt, dt, U,
               W1, b1, g1, be1, m1, v1,
               W2, b2, g2, be2, m2, v2,
               W3, b3, Wnu, bnu, WB, bB, WC, bC, WD, bD):
    f32 = np.float32
    s1 = (g1 / np.sqrt(v1 + EPS)).astype(f32)
    t1 = ((b1 - m1) * s1 + be1).astype(f32)
    s2 = (g2 / np.sqrt(v2 + EPS)).astype(f32)
    t2 = ((b2 - m2) * s2 + be2).astype(f32)

    # mm3: lhsT = [WnuF | W3] -> psum rows 0:64 nu-pre, 64:128 hz-pre
    # nu = hz @ Wnu + bnu with hz = h2 @ W3 + b3, so fold through W3:
    # nu = h2 @ (W3 @ Wnu) + (b3 @ Wnu + bnu)
    WnuF = (np.asarray(W3, f32) @ np.asarray(Wnu, f32)).astype(f32)
    bnuF = (np.asarray(b3, f32) @ np.asarray(Wnu, f32) + np.asarray(bnu, f32)).astype(f32)
    W3nu = np.concatenate([WnuF, np.asarray(W3, f32)], axis=1)  # [128, 128]

    # G weights: row (u*64 + k); cols [n (64) | y (50)]
    WBr = np.asarray(WB, f32).reshape(LATENT, LATENT, UDIM)   # [k, n, u]
    WDr = np.asarray(WD, f32).reshape(LATENT, NY, UDIM)       # [k, y, u]
    WBDp = np.zeros((UDIM * LATENT, LATENT + NY), f32)
    WBDp[:, :LATENT] = WBr.transpose(2, 0, 1).reshape(UDIM * LATENT, LATENT)
    WBDp[:, LATENT:] = WDr.transpose(2, 0, 1).reshape(UDIM * LATENT, NY)
    # bias rows contracted with udt: [16, 114]
    bBr = np.asarray(bB, f32).reshape(LATENT, UDIM)           # [n, u]
    bDr = np.asarray(bD, f32).reshape(NY, UDIM)               # [y, u]
    WBDb = np.concatenate([bBr.T, bDr.T], axis=1)             # [16, 114]

    # G2 weights: row (n*64 + k), col y
    WCr = np.asarray(WC, f32).reshape(LATENT, NY, LATENT)     # [k, y, n]
    WCp = WCr.transpose(2, 0, 1).reshape(LATENT * LATENT, NY) # [(n,k), y]
    bCr = np.asarray(bC, f32).reshape(NY, LATENT).T           # [n, y]

    # selector matrices
    selG = np.zeros((UDIM, NUC * 128), f32)
    for c in range(NUC):
        for m in range(128):
            selG[2 * c + m // 64, 128 * c + m] = 1.0
    selZ = np.zeros((LATENT, NKC * 128), f32)
    for c in range(NKC):
        for m in range(128):
            selZ[2 * c + m // 64, 128 * c + m] = 1.0
    IDup = np.zeros((LATENT, 128), f32)
    for m in range(128):
        IDup[m % 64, m] = 1.0

    # lhsT chunk layout for wide weights: [128, nchunk*cols]
    WBDp_s = np.concatenate(
        [WBDp[128 * c:128 * (c + 1), :] for c in range(8)], axis=1)   # [128, 8*114]
    WCp_s = np.concatenate(
        [WCp[128 * c:128 * (c + 1), :] for c in range(32)], axis=1)   # [128, 32*50]

    # transposed streaming inputs
    zT0 = np.ascontiguousarray(np.asarray(zt, f32).T)                  # [64, B]
    Ut = np.asarray(U, f32)
    dtv = np.asarray(dt, f32)                                          # [B, 1]
    UX = np.concatenate(
        [Ut.transpose(0, 2, 1),                                        # [T, 16, B]
         np.broadcast_to(dtv.T[None], (T, 1, B))], axis=1)             # [T, 17, B]
    UX = np.ascontiguousarray(UX, f32)
    UDT = np.ascontiguousarray((Ut * dtv[None]).transpose(0, 2, 1), f32)  # [T, 16, B]

    per_core_const = {
        "W1": np.ascontiguousarray(np.asarray(W1, f32)),
        "W2": np.ascontiguousarray(np.asarray(W2, f32)),
        "W3nu": np.ascontiguousarray(W3nu),
        "s1": s1.reshape(H1, 1), "t1": t1.reshape(H1, 1),
        "s2": s2.reshape(H2, 1), "t2": t2.reshape(H2, 1),
        "b3": np.asarray(b3, f32).reshape(LATENT, 1),
        "bnuF": bnuF.reshape(LATENT, 1),
        "WBDp": np.ascontiguousarray(WBDp_s),
        "WBDb": np.ascontiguousarray(WBDb),
        "WCp": np.ascontiguousarray(WCp_s),
        "bCr": np.ascontiguousarray(bCr),
        "selG": selG, "selZ": selZ, "IDup": IDup,
    }
    return per_core_const, zT0, UX, UDT


def _build_program():
    from concourse import bacc, mybir, tile
    from concourse import bass as cbass

    f32 = mybir.dt.float32
    mmdt = getattr(mybir.dt, MM_DT)
    Act = mybir.ActivationFunctionType
    Alu = mybir.AluOpType

    nc = bacc.Bacc("TRN2", target_bir_lowering=False, debug=False, num_devices=1)

    def din(name, shape):
        return nc.dram_tensor(name, list(shape), f32, kind="ExternalInput").ap()

    zT0 = din("zT0", [LATENT, BS])
    UX = din("UX", [T, DIN - LATENT, BS])
    UDT = din("UDT", [T, UDIM, BS])
    W1 = din("W1", [DIN, H1])
    W2 = din("W2", [H1, H2])
    W3nu = din("W3nu", [H2, 128])
    s1 = din("s1", [H1, 1]); t1 = din("t1", [H1, 1])
    s2 = din("s2", [H2, 1]); t2 = din("t2", [H2, 1])
    b3 = din("b3", [LATENT, 1]); bnuF = din("bnuF", [LATENT, 1])
    WBDp = din("WBDp", [128, 8 * (LATENT + NY)])
    WBDb = din("WBDb", [UDIM, LATENT + NY])
    WCp = din("WCp", [128, 32 * NY])
    bCr = din("bCr", [LATENT, NY])
    selG = din("selG", [UDIM, NUC * 128])
    selZ = din("selZ", [LATENT, NKC * 128])
    IDup = din("IDup", [LATENT, 128])

    ZT = nc.dram_tensor("ZT", [T, LATENT, BS], f32, kind="ExternalOutput").ap()
    YT = nc.dram_tensor("YT", [T, NY, BS], f32, kind="ExternalOutput").ap()

    NW = LATENT + NY  # 114

    with tile.TileContext(nc) as tc:
        with tc.tile_pool(name="const", bufs=1) as cpool, \
             tc.tile_pool(name="x", bufs=3) as xpool, \
             tc.tile_pool(name="h", bufs=3) as hpool, \
             tc.tile_pool(name="hz2", bufs=2) as hz2pool, \
             tc.tile_pool(name="small", bufs=4) as spool, \
             tc.tile_pool(name="g", bufs=4) as gpool, \
             tc.tile_pool(name="y", bufs=3) as ypool, \
             tc.tile_pool(name="udt", bufs=3) as upool, \
             tc.tile_pool(name="psmlp", bufs=3, space="PSUM") as psmlp, \
             tc.tile_pool(name="psbc", bufs=2, space="PSUM") as psbc, \
             tc.tile_pool(name="psA", bufs=1, space="PSUM") as psApool, \
             tc.tile_pool(name="psB", bufs=1, space="PSUM") as psBpool:

            def load_const(src, shape, tag):
                t = cpool.tile(list(shape), f32, tag=tag)
                nc.sync.dma_start(out=t[:, :], in_=src)
                return t

            W1s = load_const(W1, [DIN, H1], "cW1")
            W2s = load_const(W2, [H1, H2], "cW2")
            W3nus = load_const(W3nu, [H2, 128], "cW3nu")
            s1s = load_const(s1, [H1, 1], "cs1"); t1s = load_const(t1, [H1, 1], "ct1")
            s2s = load_const(s2, [H2, 1], "cs2"); t2s = load_const(t2, [H2, 1], "ct2")
            b3s = load_const(b3, [LATENT, 1], "cb3")
            bnus = load_const(bnuF, [LATENT, 1], "cbnu")
            WBDps = load_const(WBDp, [128, 8 * NW], "cWBDp")
            WBDbs = load_const(WBDb, [UDIM, NW], "cWBDb")
            WCps = load_const(WCp, [128, 32 * NY], "cWCp")
            bCrs = load_const(bCr, [LATENT, NY], "cbCr")
            selGs = load_const(selG, [UDIM, NUC * 128], "cselG")
            selZs = load_const(selZ, [LATENT, NKC * 128], "cselZ")
            IDups = load_const(IDup, [LATENT, 128], "cIDup")

            def mmr(ap):
                return ap.bitcast(mmdt) if MM_DT == "float32r" else ap

            # initial x tile: [z; u; dt]
            x_prev = xpool.tile([DIN, BS], f32)
            nc.sync.dma_start(out=x_prev[0:LATENT, :], in_=zT0)
            nc.sync.dma_start(out=x_prev[LATENT:DIN, :], in_=UX[0])

            for t in range(T):
                # ---- MLP ----
                ps1 = psmlp.tile([H1, BS], f32, tag="ps")
                nc.tensor.matmul(ps1[:, :], mmr(W1s[:, :]), mmr(x_prev[:, :]),
                                 start=True, stop=True)
                h1 = hpool.tile([H1, BS], f32, tag="h")
                nc.scalar.activation(h1[:, :], ps1[:, :], Act.Relu,
                                     bias=t1s[:, :], scale=s1s[:, :])

                ps2 = psmlp.tile([H2, BS], f32, tag="ps")
                nc.tensor.matmul(ps2[:, :], mmr(W2s[:, :]), mmr(h1[:, :]),
                                 start=True, stop=True)
                h2 = hpool.tile([H2, BS], f32, tag="h")
                nc.scalar.activation(h2[:, :], ps2[:, :], Act.Relu,
                                     bias=t2s[:, :], scale=s2s[:, :])

                ps3 = psmlp.tile([128, BS], f32, tag="ps")
                nc.tensor.matmul(ps3[:, :], mmr(W3nus[:, :]), mmr(h2[:, :]),
                                 start=True, stop=True)

                # HZ2 = [hz; hz]
                HZ2 = hz2pool.tile([128, BS], f32)
                nc.scalar.activation(HZ2[0:LATENT, :], ps3[LATENT:128, :],
                                     Act.Identity, bias=b3s[:, :])
                psd = psmlp.tile([128, BS], f32, tag="ps")
                nc.tensor.matmul(psd[:, :], mmr(IDups[:, :]), mmr(HZ2[0:LATENT, :]),
                                 start=True, stop=True)
                nc.scalar.activation(HZ2[LATENT:128, :], psd[LATENT:128, :],
                                     Act.Identity, bias=0.0)

                # Lam = exp(-exp(min(nu + bnuF, 2)))
                nuc = spool.tile([LATENT, BS], f32, tag="nu")
                nc.vector.tensor_scalar(nuc[:, :], ps3[0:LATENT, :],
                                        bnus[:, :], NU_MAX, Alu.add, Alu.min)
                ex = spool.tile([LATENT, BS], f32, tag="ex")
                nc.scalar.activation(ex[:, :], nuc[:, :], Act.Exp)
                Lam = spool.tile([LATENT, BS], f32, tag="lam")
                nc.scalar.activation(Lam[:, :], ex[:, :], Act.Exp, scale=-1.0)

                # ---- G path: psA = WBD.T @ (udt (x) hz) + bias rows ----
                udt_t = upool.tile([UDIM, BS], f32)
                nc.sync.dma_start(out=udt_t[:, :], in_=UDT[t])

                psA = psApool.tile([NW, BS], f32)
                for c in range(NUC):
                    bc = psbc.tile([128, BS], f32, tag="bc")
                    nc.tensor.matmul(bc[:, :],
                                     mmr(selGs[:, 128 * c:128 * (c + 1)]),
                                     mmr(udt_t[:, :]), start=True, stop=True)
                    g = gpool.tile([128, BS], f32, tag="g")
                    nc.vector.tensor_tensor(g[:, :], bc[:, :], HZ2[:, :], Alu.mult)
                    nc.tensor.matmul(psA[:, :],
                                     mmr(WBDps[:, NW * c:NW * (c + 1)]),
                                     mmr(g[:, :]), start=(c == 0), stop=False)
                nc.tensor.matmul(psA[:, :], mmr(WBDbs[:, :]), mmr(udt_t[:, :]),
                                 start=False, stop=True)

                # ---- z1 = Lam * z + psA[0:64] ----
                zl = spool.tile([LATENT, BS], f32, tag="zl")
                nc.vector.tensor_tensor(zl[:, :], Lam[:, :], x_prev[0:LATENT, :],
                                        Alu.mult)
                x_next = xpool.tile([DIN, BS], f32)
                nc.vector.tensor_tensor(x_next[0:LATENT, :], zl[:, :],
                                        psA[0:LATENT, :], Alu.add)
                nc.sync.dma_start(out=ZT[t], in_=x_next[0:LATENT, :])
                if t + 1 < T:
                    nc.sync.dma_start(out=x_next[LATENT:DIN, :], in_=UX[t + 1])

                # ---- G2 path: psB = WCp.T @ (z1 (x) hz) + bCr.T @ z1 ----
                psB = psBpool.tile([NY, BS], f32)
                for c in range(NKC):
                    bc = psbc.tile([128, BS], f32, tag="bc")
                    nc.tensor.matmul(bc[:, :],
                                     mmr(selZs[:, 128 * c:128 * (c + 1)]),
                                     mmr(x_next[0:LATENT, :]), start=True, stop=True)
                    g = gpool.tile([128, BS], f32, tag="g")
                    nc.vector.tensor_tensor(g[:, :], bc[:, :], HZ2[:, :], Alu.mult)
                    nc.tensor.matmul(psB[:, :],
                                     mmr(WCps[:, NY * c:NY * (c + 1)]),
                                     mmr(g[:, :]), start=(c == 0), stop=False)
                nc.tensor.matmul(psB[:, :], mmr(bCrs[:, :]),
                                 mmr(x_next[0:LATENT, :]), start=False, stop=True)

                # ---- y = psA[64:114] + psB ----
                yb = ypool.tile([NY, BS], f32, tag="yb")
                nc.scalar.activation(yb[:, :], psB[:, :], Act.Identity, bias=0.0)
                yt = ypool.tile([NY, BS], f32, tag="yt")
                nc.vector.tensor_tensor(yt[:, :], yb[:, :], psA[LATENT:NW, :],
                                        Alu.add)
                nc.sync.dma_start(out=YT[t], in_=yt[:, :])

                x_prev = x_next

    nc.compile()
    return nc


_NC_CACHE = None
LAST_RESULT = None


def kernel(**inputs):
    global _NC_CACHE, LAST_RESULT
    import os
    from concourse.bass_utils import run_bass_kernel_spmd

    const, zT0, UX, UDT = _host_prep(**inputs)

    if _NC_CACHE is None:
        _NC_CACHE = _build_program()
    nc = _NC_CACHE

    in_maps = []
    for i in range(NCORES):
        sl = slice(i * BS, (i + 1) * BS)
        m = dict(const)
        m["zT0"] = np.ascontiguousarray(zT0[:, sl])
        m["UX"] = np.ascontiguousarray(UX[:, :, sl])
        m["UDT"] = np.ascontiguousarray(UDT[:, :, sl])
        in_maps.append(m)

    res = run_bass_kernel_spmd(nc, in_maps, core_ids=list(range(NCORES)),
                               trace=bool(os.environ.get("KERNEL_TRACE")))
    LAST_RESULT = res

    Z = np.empty((T, B, LATENT), np.float32)
    Y = np.empty((T, B, NY), np.float32)
    for i in range(NCORES):
        sl = slice(i * BS, (i + 1) * BS)
        Z[:, sl, :] = res.results[i]["ZT"].transpose(0, 2, 1)
        Y[:, sl, :] = res.results[i]["YT"].transpose(0, 2, 1)
    return Z, Y


# revision 6
# speedup vs baseline: 1.5893x; 1.5893x over previous
"""Trainium2 Bass kernel for the CLRU transition model.

Data-parallel over batch: 8 cores x 256 batch each. T=128 sequential steps.
Activations live transposed in SBUF: [feature-partitions, batch-free].

Per step (per core), with zT [64,256] state and hzT [64,256]:
  h1 = relu((x@W1)*s1 + t1)        x = [z; u; dt] (K=81)
  h2 = relu((h1@W2)*s2 + t2)
  ps3 = [Wnu | W3] contraction     -> nu-pre rows 0:64, hz-pre rows 64:128
  Lam = exp(-exp(min(nu + bnu, 2)))
  z1add/y_d = WBDp.T @ G           G = udt (x) hz outer product  [1024+16, 256]
  y_c       = WCp.T  @ G2          G2 = z1  (x) hz outer product [4096+64, 256]
  z1 = Lam*z + z1add ; y = y_d + y_c

Outer products are built chunkwise (128 rows): a selector matmul broadcasts
pairs of z1/udt rows across partitions into PSUM, then DVE multiplies by
HZ2 = [hz; hz].  Bias terms bB/bD/bC are folded in as extra contraction
chunks whose rhs are udtT / z1T themselves.
"""

import numpy as np

LATENT, UDIM, NY = 64, 16, 50
H1, H2 = 128, 128
B, T = 2048, 128
DIN = LATENT + UDIM + 1
EPS = 1e-5
NU_MAX = 2.0
NCORES = 8
BS = B // NCORES          # 256 batch per core
NKC = LATENT // 2         # 32 G2 chunks (2 n-rows x 64 k each)
NUC = UDIM // 2           # 8 G chunks

MM_DT = "float32r"        # matmul dtype ("float32" exact, "float32r" fast/TF32-like)
ELEM_DT = "float32"       # dtype of outer-product chunks fed to big matmuls


def _host_prep(zt, dt, U,
               W1, b1, g1, be1, m1, v1,
               W2, b2, g2, be2, m2, v2,
               W3, b3, Wnu, bnu, WB, bB, WC, bC, WD, bD):
    f32 = np.float32
    s1 = (g1 / np.sqrt(v1 + EPS)).astype(f32)
    t1 = ((b1 - m1) * s1 + be1).astype(f32)
    s2 = (g2 / np.sqrt(v2 + EPS)).astype(f32)
    t2 = ((b2 - m2) * s2 + be2).astype(f32)

    # mm3: lhsT = [WnuF | W3] -> psum rows 0:64 nu-pre, 64:128 hz-pre
    # nu = hz @ Wnu + bnu with hz = h2 @ W3 + b3, so fold through W3:
    # nu = h2 @ (W3 @ Wnu) + (b3 @ Wnu + bnu)
    WnuF = (np.asarray(W3, f32) @ np.asarray(Wnu, f32)).astype(f32)
    bnuF = (np.asarray(b3, f32) @ np.asarray(Wnu, f32) + np.asarray(bnu, f32)).astype(f32)
    W3nu = np.concatenate([WnuF, np.asarray(W3, f32)], axis=1)  # [128, 128]

    # G weights: row (u*64 + k); cols [n (64) | y (50)]
    WBr = np.asarray(WB, f32).reshape(LATENT, LATENT, UDIM)   # [k, n, u]
    WDr = np.asarray(WD, f32).reshape(LATENT, NY, UDIM)       # [k, y, u]
    WBDp = np.zeros((UDIM * LATENT, LATENT + NY), f32)
    WBDp[:, :LATENT] = WBr.transpose(2, 0, 1).reshape(UDIM * LATENT, LATENT)
    WBDp[:, LATENT:] = WDr.transpose(2, 0, 1).reshape(UDIM * LATENT, NY)
    # bias rows contracted with udt: [16, 114]
    bBr = np.asarray(bB, f32).reshape(LATENT, UDIM)           # [n, u]
    bDr = np.asarray(bD, f32).reshape(NY, UDIM)               # [y, u]
    WBDb = np.concatenate([bBr.T, bDr.T], axis=1)             # [16, 114]

    # G2 weights: row (n*64 + k), col y
    WCr = np.asarray(WC, f32).reshape(LATENT, NY, LATENT)     # [k, y, n]
    WCp = WCr.transpose(2, 0, 1).reshape(LATENT * LATENT, NY) # [(n,k), y]
    bCr = np.asarray(bC, f32).reshape(NY, LATENT).T           # [n, y]

    # selector matrices
    selG = np.zeros((UDIM, NUC * 128), f32)
    for c in range(NUC):
        for m in range(128):
            selG[2 * c + m // 64, 128 * c + m] = 1.0
    selZ = np.zeros((LATENT, NKC * 128), f32)
    for c in range(NKC):
        for m in range(128):
            selZ[2 * c + m // 64, 128 * c + m] = 1.0
    IDup = np.zeros((LATENT, 128), f32)
    for m in range(128):
        IDup[m % 64, m] = 1.0

    # lhsT chunk layout for wide weights: [128, nchunk*cols]
    WBDp_s = np.concatenate(
        [WBDp[128 * c:128 * (c + 1), :] for c in range(8)], axis=1)   # [128, 8*114]
    WCp_s = np.concatenate(
        [WCp[128 * c:128 * (c + 1), :] for c in range(32)], axis=1)   # [128, 32*50]

    # transposed streaming inputs
    zT0 = np.ascontiguousarray(np.asarray(zt, f32).T)                  # [64, B]
    Ut = np.asarray(U, f32)
    dtv = np.asarray(dt, f32)                                          # [B, 1]
    UX = np.concatenate(
        [Ut.transpose(0, 2, 1),                                        # [T, 16, B]
         np.broadcast_to(dtv.T[None], (T, 1, B))], axis=1)             # [T, 17, B]
    UX = np.ascontiguousarray(UX, f32)
    UDT = np.ascontiguousarray((Ut * dtv[None]).transpose(0, 2, 1), f32)  # [T, 16, B]

    per_core_const = {
        "W1": np.ascontiguousarray(np.asarray(W1, f32)),
        "W2": np.ascontiguousarray(np.asarray(W2, f32)),
        "W3nu": np.ascontiguousarray(W3nu),
        "s1": s1.reshape(H1, 1), "t1": t1.reshape(H1, 1),
        "s2": s2.reshape(H2, 1), "t2": t2.reshape(H2, 1),
        "b3": np.asarray(b3, f32).reshape(LATENT, 1),
        "bnuF": bnuF.reshape(LATENT, 1),
        "WBDp": np.ascontiguousarray(WBDp_s),
        "WBDb": np.ascontiguousarray(WBDb),
        "WCp": np.ascontiguousarray(WCp_s),
        "bCr": np.ascontiguousarray(bCr),
        "selG": selG, "selZ": selZ, "IDup": IDup,
    }
    return per_core_const, zT0, UX, UDT


def _build_program():
    from concourse import bacc, mybir, tile
    from concourse import bass as cbass

    f32 = mybir.dt.float32
    mmdt = getattr(mybir.dt, MM_DT)
    Act = mybir.ActivationFunctionType
    Alu = mybir.AluOpType

    nc = bacc.Bacc("TRN2", target_bir_lowering=False, debug=False, num_devices=1)

    def din(name, shape, dt=None):
        return nc.dram_tensor(name, list(shape), dt or mmdt, kind="ExternalInput").ap()

    zT0 = din("zT0", [LATENT, BS])
    UX = din("UX", [T, DIN - LATENT, BS])
    UDT = din("UDT", [T, UDIM, BS])
    W1 = din("W1", [DIN, H1])
    W2 = din("W2", [H1, H2])
    W3nu = din("W3nu", [H2, 128])
    s1 = din("s1", [H1, 1], f32); t1 = din("t1", [H1, 1], f32)
    s2 = din("s2", [H2, 1], f32); t2 = din("t2", [H2, 1], f32)
    b3 = din("b3", [LATENT, 1], f32); bnuF = din("bnuF", [LATENT, 1], f32)
    WBDp = din("WBDp", [128, 8 * (LATENT + NY)])
    WBDb = din("WBDb", [UDIM, LATENT + NY])
    WCp = din("WCp", [128, 32 * NY])
    bCr = din("bCr", [LATENT, NY])
    selG = din("selG", [UDIM, NUC * 128])
    selZ = din("selZ", [LATENT, NKC * 128])
    IDup = din("IDup", [LATENT, 128])

    ZT = nc.dram_tensor("ZT", [T, LATENT, BS], mmdt, kind="ExternalOutput").ap()
    YT = nc.dram_tensor("YT", [T, NY, BS], f32, kind="ExternalOutput").ap()

    NW = LATENT + NY  # 114

    with tile.TileContext(nc) as tc:
        with tc.tile_pool(name="const", bufs=1) as cpool, \
             tc.tile_pool(name="x", bufs=3) as xpool, \
             tc.tile_pool(name="h", bufs=3) as hpool, \
             tc.tile_pool(name="hz2", bufs=2) as hz2pool, \
             tc.tile_pool(name="small", bufs=4) as spool, \
             tc.tile_pool(name="g", bufs=4) as gpool, \
             tc.tile_pool(name="y", bufs=3) as ypool, \
             tc.tile_pool(name="udt", bufs=3) as upool, \
             tc.tile_pool(name="psmlp", bufs=3, space="PSUM") as psmlp, \
             tc.tile_pool(name="psbc", bufs=2, space="PSUM") as psbc, \
             tc.tile_pool(name="psA", bufs=1, space="PSUM") as psApool, \
             tc.tile_pool(name="psB", bufs=1, space="PSUM") as psBpool:

            def load_const(src, shape, tag, dt=None):
                t = cpool.tile(list(shape), dt or mmdt, tag=tag)
                nc.sync.dma_start(out=t[:, :], in_=src)
                return t

            W1s = load_const(W1, [DIN, H1], "cW1")
            W2s = load_const(W2, [H1, H2], "cW2")
            W3nus = load_const(W3nu, [H2, 128], "cW3nu")
            s1s = load_const(s1, [H1, 1], "cs1", f32); t1s = load_const(t1, [H1, 1], "ct1", f32)
            s2s = load_const(s2, [H2, 1], "cs2", f32); t2s = load_const(t2, [H2, 1], "ct2", f32)
            b3s = load_const(b3, [LATENT, 1], "cb3", f32)
            bnus = load_const(bnuF, [LATENT, 1], "cbnu", f32)
            WBDps = load_const(WBDp, [128, 8 * NW], "cWBDp")
            WBDbs = load_const(WBDb, [UDIM, NW], "cWBDb")
            WCps = load_const(WCp, [128, 32 * NY], "cWCp")
            bCrs = load_const(bCr, [LATENT, NY], "cbCr")
            selGs = load_const(selG, [UDIM, NUC * 128], "cselG")
            selZs = load_const(selZ, [LATENT, NKC * 128], "cselZ")
            IDups = load_const(IDup, [LATENT, 128], "cIDup")

            def mmr(ap):
                return ap

            # initial x tile: [z; u; dt]
            x_prev = xpool.tile([DIN, BS], mmdt)
            nc.sync.dma_start(out=x_prev[0:LATENT, :], in_=zT0)
            nc.sync.dma_start(out=x_prev[LATENT:DIN, :], in_=UX[0])

            for t in range(T):
                # ---- MLP ----
                ps1 = psmlp.tile([H1, BS], f32, tag="ps")
                nc.tensor.matmul(ps1[:, :], mmr(W1s[:, :]), mmr(x_prev[:, :]),
                                 start=True, stop=True)
                h1 = hpool.tile([H1, BS], mmdt, tag="h")
                nc.scalar.activation(h1[:, :], ps1[:, :], Act.Relu,
                                     bias=t1s[:, :], scale=s1s[:, :])

                ps2 = psmlp.tile([H2, BS], f32, tag="ps")
                nc.tensor.matmul(ps2[:, :], mmr(W2s[:, :]), mmr(h1[:, :]),
                                 start=True, stop=True)
                h2 = hpool.tile([H2, BS], mmdt, tag="h")
                nc.scalar.activation(h2[:, :], ps2[:, :], Act.Relu,
                                     bias=t2s[:, :], scale=s2s[:, :])

                ps3 = psmlp.tile([128, BS], f32, tag="ps")
                nc.tensor.matmul(ps3[:, :], mmr(W3nus[:, :]), mmr(h2[:, :]),
                                 start=True, stop=True)

                # HZ2 = [hz; hz]
                HZ2 = hz2pool.tile([128, BS], mmdt)
                nc.scalar.activation(HZ2[0:LATENT, :], ps3[LATENT:128, :],
                                     Act.Identity, bias=b3s[:, :])
                psd = psmlp.tile([128, BS], f32, tag="ps")
                nc.tensor.matmul(psd[:, :], mmr(IDups[:, :]), mmr(HZ2[0:LATENT, :]),
                                 start=True, stop=True)
                nc.scalar.activation(HZ2[LATENT:128, :], psd[LATENT:128, :],
                                     Act.Identity, bias=0.0)

                # Lam = exp(-exp(min(nu + bnuF, 2)))
                nuc = spool.tile([LATENT, BS], f32, tag="nu")
                nc.vector.tensor_scalar(nuc[:, :], ps3[0:LATENT, :],
                                        bnus[:, :], NU_MAX, Alu.add, Alu.min)
                ex = spool.tile([LATENT, BS], f32, tag="ex")
                nc.scalar.activation(ex[:, :], nuc[:, :], Act.Exp)
                Lam = spool.tile([LATENT, BS], f32, tag="lam")
                nc.scalar.activation(Lam[:, :], ex[:, :], Act.Exp, scale=-1.0)

                # ---- G path: psA = WBD.T @ (udt (x) hz) + bias rows ----
                udt_t = upool.tile([UDIM, BS], mmdt)
                nc.sync.dma_start(out=udt_t[:, :], in_=UDT[t])

                psA = psApool.tile([NW, BS], f32)
                for c in range(NUC):
                    bc = psbc.tile([128, BS], f32, tag="bc")
                    nc.tensor.matmul(bc[:, :],
                                     mmr(selGs[:, 128 * c:128 * (c + 1)]),
                                     mmr(udt_t[:, :]), start=True, stop=True)
                    g = gpool.tile([128, BS], mmdt, tag="g")
                    nc.vector.tensor_tensor(g[:, :], bc[:, :], HZ2[:, :], Alu.mult)
                    nc.tensor.matmul(psA[:, :],
                                     mmr(WBDps[:, NW * c:NW * (c + 1)]),
                                     mmr(g[:, :]), start=(c == 0), stop=False)
                nc.tensor.matmul(psA[:, :], mmr(WBDbs[:, :]), mmr(udt_t[:, :]),
                                 start=False, stop=True)

                # ---- z1 = Lam * z + psA[0:64] ----
                zl = spool.tile([LATENT, BS], f32, tag="zl")
                nc.vector.tensor_tensor(zl[:, :], Lam[:, :], x_prev[0:LATENT, :],
                                        Alu.mult)
                x_next = xpool.tile([DIN, BS], mmdt)
                nc.vector.tensor_tensor(x_next[0:LATENT, :], zl[:, :],
                                        psA[0:LATENT, :], Alu.add)
                nc.sync.dma_start(out=ZT[t], in_=x_next[0:LATENT, :])
                if t + 1 < T:
                    nc.sync.dma_start(out=x_next[LATENT:DIN, :], in_=UX[t + 1])

                # ---- G2 path: psB = WCp.T @ (z1 (x) hz) + bCr.T @ z1 ----
                psB = psBpool.tile([NY, BS], f32)
                for c in range(NKC):
                    bc = psbc.tile([128, BS], f32, tag="bc")
                    nc.tensor.matmul(bc[:, :],
                                     mmr(selZs[:, 128 * c:128 * (c + 1)]),
                                     mmr(x_next[0:LATENT, :]), start=True, stop=True)
                    g = gpool.tile([128, BS], mmdt, tag="g")
                    nc.vector.tensor_tensor(g[:, :], bc[:, :], HZ2[:, :], Alu.mult)
                    nc.tensor.matmul(psB[:, :],
                                     mmr(WCps[:, NY * c:NY * (c + 1)]),
                                     mmr(g[:, :]), start=(c == 0), stop=False)
                nc.tensor.matmul(psB[:, :], mmr(bCrs[:, :]),
                                 mmr(x_next[0:LATENT, :]), start=False, stop=True)

                # ---- y = psA[64:114] + psB ----
                yb = ypool.tile([NY, BS], f32, tag="yb")
                nc.scalar.activation(yb[:, :], psB[:, :], Act.Identity, bias=0.0)
                yt = ypool.tile([NY, BS], f32, tag="yt")
                nc.vector.tensor_tensor(yt[:, :], yb[:, :], psA[LATENT:NW, :],
                                        Alu.add)
                nc.sync.dma_start(out=YT[t], in_=yt[:, :])

                x_prev = x_next

    nc.compile()
    return nc


_NC_CACHE = None
LAST_RESULT = None


def kernel(**inputs):
    global _NC_CACHE, LAST_RESULT
    import os
    from concourse.bass_utils import run_bass_kernel_spmd

    const, zT0, UX, UDT = _host_prep(**inputs)

    if _NC_CACHE is None:
        _NC_CACHE = _build_program()
    nc = _NC_CACHE

    in_maps = []
    for i in range(NCORES):
        sl = slice(i * BS, (i + 1) * BS)
        m = dict(const)
        m["zT0"] = np.ascontiguousarray(zT0[:, sl])
        m["UX"] = np.ascontiguousarray(UX[:, :, sl])
        m["UDT"] = np.ascontiguousarray(UDT[:, :, sl])
        in_maps.append(m)

    res = run_bass_kernel_spmd(nc, in_maps, core_ids=list(range(NCORES)),
                               trace=bool(os.environ.get("KERNEL_TRACE")))
    LAST_RESULT = res

    Z = np.empty((T, B, LATENT), np.float32)
    Y = np.empty((T, B, NY), np.float32)
    for i in range(NCORES):
        sl = slice(i * BS, (i + 1) * BS)
        Z[:, sl, :] = res.results[i]["ZT"].transpose(0, 2, 1)
        Y[:, sl, :] = res.results[i]["YT"].transpose(0, 2, 1)
    return Z, Y


# revision 7
# speedup vs baseline: 2.3631x; 1.4869x over previous
"""Trainium2 Bass kernel for the CLRU transition model.

Data-parallel over batch: 8 cores x 256 batch each. T=128 sequential steps.
Activations live transposed in SBUF: [feature-partitions, batch-free].

Per step (per core), with zT [64,256] state and hzT [64,256]:
  h1 = relu((x@W1)*s1 + t1)        x = [z; u; dt] (K=81)
  h2 = relu((h1@W2)*s2 + t2)
  ps3 = [Wnu | W3] contraction     -> nu-pre rows 0:64, hz-pre rows 64:128
  Lam = exp(-exp(min(nu + bnu, 2)))
  z1add/y_d = WBDp.T @ G           G = udt (x) hz outer product  [1024+16, 256]
  y_c       = WCp.T  @ G2          G2 = z1  (x) hz outer product [4096+64, 256]
  z1 = Lam*z + z1add ; y = y_d + y_c

Outer products are built chunkwise (128 rows): a selector matmul broadcasts
pairs of z1/udt rows across partitions into PSUM, then DVE multiplies by
HZ2 = [hz; hz].  Bias terms bB/bD/bC are folded in as extra contraction
chunks whose rhs are udtT / z1T themselves.
"""

import numpy as np

LATENT, UDIM, NY = 64, 16, 50
H1, H2 = 128, 128
B, T = 2048, 128
DIN = LATENT + UDIM + 1
EPS = 1e-5
NU_MAX = 2.0
NCORES = 8
BS = B // NCORES          # 256 batch per core
NKC = LATENT // 2         # 32 G2 chunks (2 n-rows x 64 k each)
NUC = UDIM // 2           # 8 G chunks

MM_DT = "float32r"        # matmul dtype ("float32" exact, "float32r" fast/TF32-like)
ELEM_DT = "float32"       # dtype of outer-product chunks fed to big matmuls


def _host_prep(zt, dt, U,
               W1, b1, g1, be1, m1, v1,
               W2, b2, g2, be2, m2, v2,
               W3, b3, Wnu, bnu, WB, bB, WC, bC, WD, bD):
    f32 = np.float32
    s1 = (g1 / np.sqrt(v1 + EPS)).astype(f32)
    t1 = ((b1 - m1) * s1 + be1).astype(f32)
    s2 = (g2 / np.sqrt(v2 + EPS)).astype(f32)
    t2 = ((b2 - m2) * s2 + be2).astype(f32)

    # mm3: lhsT = [WnuF | W3] -> psum rows 0:64 nu-pre, 64:128 hz-pre
    # nu = hz @ Wnu + bnu with hz = h2 @ W3 + b3, so fold through W3:
    # nu = h2 @ (W3 @ Wnu) + (b3 @ Wnu + bnu)
    WnuF = (np.asarray(W3, f32) @ np.asarray(Wnu, f32)).astype(f32)
    bnuF = (np.asarray(b3, f32) @ np.asarray(Wnu, f32) + np.asarray(bnu, f32)).astype(f32)
    W3nu = np.concatenate([WnuF, np.asarray(W3, f32)], axis=1)  # [128, 128]

    # G weights: row (u*64 + k); cols [n (64) | y (50)]
    WBr = np.asarray(WB, f32).reshape(LATENT, LATENT, UDIM)   # [k, n, u]
    WDr = np.asarray(WD, f32).reshape(LATENT, NY, UDIM)       # [k, y, u]
    WBDp = np.zeros((UDIM * LATENT, LATENT + NY), f32)
    WBDp[:, :LATENT] = WBr.transpose(2, 0, 1).reshape(UDIM * LATENT, LATENT)
    WBDp[:, LATENT:] = WDr.transpose(2, 0, 1).reshape(UDIM * LATENT, NY)
    # bias rows contracted with udt: [16, 114]
    bBr = np.asarray(bB, f32).reshape(LATENT, UDIM)           # [n, u]
    bDr = np.asarray(bD, f32).reshape(NY, UDIM)               # [y, u]
    WBDb = np.concatenate([bBr.T, bDr.T], axis=1)             # [16, 114]

    # G2 weights: row (n*64 + k), col y
    WCr = np.asarray(WC, f32).reshape(LATENT, NY, LATENT)     # [k, y, n]
    WCp = WCr.transpose(2, 0, 1).reshape(LATENT * LATENT, NY) # [(n,k), y]
    bCr = np.asarray(bC, f32).reshape(NY, LATENT).T           # [n, y]

    # selector matrices
    selZ = np.zeros((LATENT, NKC * 128), f32)
    for c in range(NKC):
        for m in range(128):
            selZ[2 * c + m // 64, 128 * c + m] = 1.0
    IDup = np.zeros((LATENT, 128), f32)
    for m in range(128):
        IDup[m % 64, m] = 1.0

    # lhsT chunk layout for wide weights: [128, nchunk*cols]
    WBDp_s = np.concatenate(
        [WBDp[128 * c:128 * (c + 1), :] for c in range(8)], axis=1)   # [128, 8*114]
    WCp_s = np.concatenate(
        [WCp[128 * c:128 * (c + 1), :] for c in range(32)], axis=1)   # [128, 32*50]

    # transposed streaming inputs
    zT0 = np.ascontiguousarray(np.asarray(zt, f32).T)                  # [64, B]
    Ut = np.asarray(U, f32)
    dtv = np.asarray(dt, f32)                                          # [B, 1]
    UX = np.concatenate(
        [Ut.transpose(0, 2, 1),                                        # [T, 16, B]
         np.broadcast_to(dtv.T[None], (T, 1, B))], axis=1)             # [T, 17, B]
    UX = np.ascontiguousarray(UX, f32)
    UDT = np.ascontiguousarray((Ut * dtv[None]).transpose(0, 2, 1), f32)  # [T, 16, B]

    per_core_const = {
        "W1": np.ascontiguousarray(np.asarray(W1, f32)),
        "W2": np.ascontiguousarray(np.asarray(W2, f32)),
        "W3nu": np.ascontiguousarray(W3nu),
        "s1": s1.reshape(H1, 1), "t1": t1.reshape(H1, 1),
        "s2": s2.reshape(H2, 1), "t2": t2.reshape(H2, 1),
        "b3": np.asarray(b3, f32).reshape(LATENT, 1),
        "bnuF": bnuF.reshape(LATENT, 1),
        "WBDp": np.ascontiguousarray(WBDp_s),
        "WBDb": np.ascontiguousarray(WBDb),
        "WCp": np.ascontiguousarray(WCp_s),
        "bCr": np.ascontiguousarray(bCr),
        "selZ": selZ, "IDup": IDup,
    }
    return per_core_const, zT0, UX, UDT


def _build_program():
    from concourse import bacc, mybir, tile
    from concourse import bass as cbass

    f32 = mybir.dt.float32
    mmdt = getattr(mybir.dt, MM_DT)
    Act = mybir.ActivationFunctionType
    Alu = mybir.AluOpType

    nc = bacc.Bacc("TRN2", target_bir_lowering=False, debug=False, num_devices=1)

    def din(name, shape, dt=None):
        return nc.dram_tensor(name, list(shape), dt or mmdt, kind="ExternalInput").ap()

    zT0 = din("zT0", [LATENT, BS])
    UX = din("UX", [T, DIN - LATENT, BS])
    UDT = din("UDT", [T, UDIM, BS])
    W1 = din("W1", [DIN, H1])
    W2 = din("W2", [H1, H2])
    W3nu = din("W3nu", [H2, 128])
    s1 = din("s1", [H1, 1], f32); t1 = din("t1", [H1, 1], f32)
    s2 = din("s2", [H2, 1], f32); t2 = din("t2", [H2, 1], f32)
    b3 = din("b3", [LATENT, 1], f32); bnuF = din("bnuF", [LATENT, 1], f32)
    WBDp = din("WBDp", [128, 8 * (LATENT + NY)])
    WBDb = din("WBDb", [UDIM, LATENT + NY])
    WCp = din("WCp", [128, 32 * NY])
    bCr = din("bCr", [LATENT, NY])
    selZ = din("selZ", [LATENT, NKC * 128])
    IDup = din("IDup", [LATENT, 128])

    ZT = nc.dram_tensor("ZT", [T, LATENT, BS], mmdt, kind="ExternalOutput").ap()
    YT = nc.dram_tensor("YT", [T, NY, BS], f32, kind="ExternalOutput").ap()

    NW = LATENT + NY  # 114

    with tile.TileContext(nc) as tc:
        with tc.tile_pool(name="const", bufs=1) as cpool, \
             tc.tile_pool(name="x", bufs=3) as xpool, \
             tc.tile_pool(name="h", bufs=3) as hpool, \
             tc.tile_pool(name="hz2", bufs=2) as hz2pool, \
             tc.tile_pool(name="small", bufs=4) as spool, \
             tc.tile_pool(name="g", bufs=3) as gpool, \
             tc.tile_pool(name="y", bufs=3) as ypool, \
             tc.tile_pool(name="udt", bufs=3) as upool, \
             tc.tile_pool(name="psmlp", bufs=2, space="PSUM") as psmlp, \
             tc.tile_pool(name="psbc", bufs=2, space="PSUM") as psbc, \
             tc.tile_pool(name="psA", bufs=1, space="PSUM") as psApool, \
             tc.tile_pool(name="psB", bufs=1, space="PSUM") as psBpool:

            def load_const(src, shape, tag, dt=None):
                t = cpool.tile(list(shape), dt or mmdt, tag=tag)
                nc.sync.dma_start(out=t[:, :], in_=src)
                return t

            W1s = load_const(W1, [DIN, H1], "cW1")
            W2s = load_const(W2, [H1, H2], "cW2")
            W3nus = load_const(W3nu, [H2, 128], "cW3nu")
            s1s = load_const(s1, [H1, 1], "cs1", f32); t1s = load_const(t1, [H1, 1], "ct1", f32)
            s2s = load_const(s2, [H2, 1], "cs2", f32); t2s = load_const(t2, [H2, 1], "ct2", f32)
            b3s = load_const(b3, [LATENT, 1], "cb3", f32)
            bnus = load_const(bnuF, [LATENT, 1], "cbnu", f32)
            WBDps = load_const(WBDp, [128, 8 * NW], "cWBDp")
            WBDbs = load_const(WBDb, [UDIM, NW], "cWBDb")
            WCps = load_const(WCp, [128, 32 * NY], "cWCp")
            bCrs = load_const(bCr, [LATENT, NY], "cbCr")
            selZs = load_const(selZ, [LATENT, NKC * 128], "cselZ")
            IDups = load_const(IDup, [LATENT, 128], "cIDup")

            def mmr(ap):
                return ap

            # initial x tile: [z; u; dt]
            x_prev = xpool.tile([DIN, BS], mmdt)
            nc.sync.dma_start(out=x_prev[0:LATENT, :], in_=zT0)
            nc.sync.dma_start(out=x_prev[LATENT:DIN, :], in_=UX[0])

            for t in range(T):
                # ---- MLP ----
                ps1 = psmlp.tile([H1, BS], f32, tag="ps")
                nc.tensor.matmul(ps1[:, :], mmr(W1s[:, :]), mmr(x_prev[:, :]),
                                 start=True, stop=True)
                h1 = hpool.tile([H1, BS], mmdt, tag="h")
                nc.scalar.activation(h1[:, :], ps1[:, :], Act.Relu,
                                     bias=t1s[:, :], scale=s1s[:, :])

                ps2 = psmlp.tile([H2, BS], f32, tag="ps")
                nc.tensor.matmul(ps2[:, :], mmr(W2s[:, :]), mmr(h1[:, :]),
                                 start=True, stop=True)
                h2 = hpool.tile([H2, BS], mmdt, tag="h")
                nc.scalar.activation(h2[:, :], ps2[:, :], Act.Relu,
                                     bias=t2s[:, :], scale=s2s[:, :])

                ps3 = psmlp.tile([128, BS], f32, tag="ps")
                nc.tensor.matmul(ps3[:, :], mmr(W3nus[:, :]), mmr(h2[:, :]),
                                 start=True, stop=True)

                # HZ2 = [hz; hz]
                HZ2 = hz2pool.tile([128, BS], mmdt)
                nc.scalar.activation(HZ2[0:LATENT, :], ps3[LATENT:128, :],
                                     Act.Identity, bias=b3s[:, :])
                psd = psmlp.tile([128, BS], f32, tag="ps")
                nc.tensor.matmul(psd[:, :], mmr(IDups[:, :]), mmr(HZ2[0:LATENT, :]),
                                 start=True, stop=True)
                nc.scalar.activation(HZ2[LATENT:128, :], psd[LATENT:128, :],
                                     Act.Identity, bias=0.0)

                # Lam = exp(-exp(min(nu + bnuF, 2)))
                nuc = spool.tile([LATENT, BS], f32, tag="nu")
                nc.vector.tensor_scalar(nuc[:, :], ps3[0:LATENT, :],
                                        bnus[:, :], NU_MAX, Alu.add, Alu.min)
                ex = spool.tile([LATENT, BS], f32, tag="ex")
                nc.scalar.activation(ex[:, :], nuc[:, :], Act.Exp)
                Lam = spool.tile([LATENT, BS], f32, tag="lam")
                nc.scalar.activation(Lam[:, :], ex[:, :], Act.Exp, scale=-1.0)

                # ---- G path: psA = WBD.T @ (udt (x) hz) + bias rows ----
                udt_t = upool.tile([UDIM, BS], mmdt, tag="udt")
                nc.sync.dma_start(out=udt_t[:, :], in_=UDT[t])
                # UdtB: chunk c cols [256c:256c+256]; rows 0:64=udt[2c], 64:128=udt[2c+1]
                ub = upool.tile([128, NUC * BS], mmdt, tag="ub")
                nc.sync.dma_start(
                    out=ub[0:64, :].rearrange("p (c f) -> p c f", c=NUC),
                    in_=UDT[t, 0:UDIM:2, :].unsqueeze(0).broadcast_to([64, NUC, BS]))
                nc.sync.dma_start(
                    out=ub[64:128, :].rearrange("p (c f) -> p c f", c=NUC),
                    in_=UDT[t, 1:UDIM:2, :].unsqueeze(0).broadcast_to([64, NUC, BS]))
                gu = gpool.tile([128, NUC * BS], mmdt, tag="gu")
                nc.gpsimd.tensor_tensor(
                    gu[:, :].rearrange("p (c f) -> p c f", c=NUC),
                    ub[:, :].rearrange("p (c f) -> p c f", c=NUC),
                    HZ2[:, :].unsqueeze(1).broadcast_to([128, NUC, BS]),
                    Alu.mult)
                psA = psApool.tile([NW, BS], f32)
                for c in range(NUC):
                    nc.tensor.matmul(psA[:, :],
                                     mmr(WBDps[:, NW * c:NW * (c + 1)]),
                                     mmr(gu[:, BS * c:BS * (c + 1)]),
                                     start=(c == 0), stop=False)
                nc.tensor.matmul(psA[:, :], mmr(WBDbs[:, :]), mmr(udt_t[:, :]),
                                 start=False, stop=True)

                # ---- z1 = Lam * z + psA[0:64] ----
                zl = spool.tile([LATENT, BS], f32, tag="zl")
                nc.gpsimd.tensor_tensor(zl[:, :], Lam[:, :], x_prev[0:LATENT, :],
                                        Alu.mult)
                x_next = xpool.tile([DIN, BS], mmdt)
                nc.vector.tensor_tensor(x_next[0:LATENT, :], zl[:, :],
                                        psA[0:LATENT, :], Alu.add)
                nc.sync.dma_start(out=ZT[t], in_=x_next[0:LATENT, :])
                if t + 1 < T:
                    nc.sync.dma_start(out=x_next[LATENT:DIN, :], in_=UX[t + 1])

                # ---- G2 path: psB = WCp.T @ (z1 (x) hz) + bCr.T @ z1 ----
                psB = psBpool.tile([NY, BS], f32)
                GRP = 4
                for grp in range(NKC // GRP):
                    mega = psbc.tile([128, GRP * BS], f32, tag="bc")
                    for j in range(GRP):
                        c = GRP * grp + j
                        nc.tensor.matmul(mega[:, BS * j:BS * (j + 1)],
                                         mmr(selZs[:, 128 * c:128 * (c + 1)]),
                                         mmr(x_next[0:LATENT, :]),
                                         start=True, stop=True)
                    g2 = gpool.tile([128, GRP * BS], mmdt, tag="g2")
                    nc.vector.tensor_tensor(
                        g2[:, :].rearrange("p (c f) -> p c f", c=GRP),
                        mega[:, :].rearrange("p (c f) -> p c f", c=GRP),
                        HZ2[:, :].unsqueeze(1).broadcast_to([128, GRP, BS]),
                        Alu.mult)
                    for j in range(GRP):
                        c = GRP * grp + j
                        nc.tensor.matmul(psB[:, :],
                                         mmr(WCps[:, NY * c:NY * (c + 1)]),
                                         mmr(g2[:, BS * j:BS * (j + 1)]),
                                         start=(c == 0), stop=False)
                nc.tensor.matmul(psB[:, :], mmr(bCrs[:, :]),
                                 mmr(x_next[0:LATENT, :]), start=False, stop=True)

                # ---- y = psA[64:114] + psB ----
                yb = ypool.tile([NY, BS], f32, tag="yb")
                nc.scalar.activation(yb[:, :], psB[:, :], Act.Identity, bias=0.0)
                yd = ypool.tile([NY, BS], f32, tag="yd")
                nc.scalar.activation(yd[:, :], psA[LATENT:NW, :], Act.Identity,
                                     bias=0.0)
                yt = ypool.tile([NY, BS], f32, tag="yt")
                nc.gpsimd.tensor_tensor(yt[:, :], yb[:, :], yd[:, :], Alu.add)
                nc.sync.dma_start(out=YT[t], in_=yt[:, :])

                x_prev = x_next

    nc.compile()
    return nc


_NC_CACHE = None
LAST_RESULT = None


def kernel(**inputs):
    global _NC_CACHE, LAST_RESULT
    import os
    from concourse.bass_utils import run_bass_kernel_spmd

    const, zT0, UX, UDT = _host_prep(**inputs)

    if _NC_CACHE is None:
        _NC_CACHE = _build_program()
    nc = _NC_CACHE

    in_maps = []
    for i in range(NCORES):
        sl = slice(i * BS, (i + 1) * BS)
        m = dict(const)
        m["zT0"] = np.ascontiguousarray(zT0[:, sl])
        m["UX"] = np.ascontiguousarray(UX[:, :, sl])
        m["UDT"] = np.ascontiguousarray(UDT[:, :, sl])
        in_maps.append(m)

    res = run_bass_kernel_spmd(nc, in_maps, core_ids=list(range(NCORES)),
                               trace=bool(os.environ.get("KERNEL_TRACE")))
    LAST_RESULT = res

    Z = np.empty((T, B, LATENT), np.float32)
    Y = np.empty((T, B, NY), np.float32)
    for i in range(NCORES):
        sl = slice(i * BS, (i + 1) * BS)
        Z[:, sl, :] = res.results[i]["ZT"].transpose(0, 2, 1)
        Y[:, sl, :] = res.results[i]["YT"].transpose(0, 2, 1)
    return Z, Y


# revision 8
# speedup vs baseline: 2.5804x; 1.0920x over previous
"""Trainium2 Bass kernel for the CLRU transition model.

Data-parallel over batch: 8 cores x 256 batch each. T=128 sequential steps.
Activations live transposed in SBUF: [feature-partitions, batch-free].

Per step (per core), with zT [64,256] state and hzT [64,256]:
  h1 = relu((x@W1)*s1 + t1)        x = [z; u; dt] (K=81)
  h2 = relu((h1@W2)*s2 + t2)
  ps3 = [Wnu | W3] contraction     -> nu-pre rows 0:64, hz-pre rows 64:128
  Lam = exp(-exp(min(nu + bnu, 2)))
  z1add/y_d = WBDp.T @ G           G = udt (x) hz outer product  [1024+16, 256]
  y_c       = WCp.T  @ G2          G2 = z1  (x) hz outer product [4096+64, 256]
  z1 = Lam*z + z1add ; y = y_d + y_c

Outer products are built chunkwise (128 rows): a selector matmul broadcasts
pairs of z1/udt rows across partitions into PSUM, then DVE multiplies by
HZ2 = [hz; hz].  Bias terms bB/bD/bC are folded in as extra contraction
chunks whose rhs are udtT / z1T themselves.
"""

import numpy as np

LATENT, UDIM, NY = 64, 16, 50
H1, H2 = 128, 128
B, T = 2048, 128
DIN = LATENT + UDIM + 1
EPS = 1e-5
NU_MAX = 2.0
NCORES = 8
BS = B // NCORES          # 256 batch per core
NKC = LATENT // 2         # 32 G2 chunks (2 n-rows x 64 k each)
NUC = UDIM // 2           # 8 G chunks

MM_DT = "float32r"        # matmul dtype ("float32" exact, "float32r" fast/TF32-like)
ELEM_DT = "float32"       # dtype of outer-product chunks fed to big matmuls


def _host_prep(zt, dt, U,
               W1, b1, g1, be1, m1, v1,
               W2, b2, g2, be2, m2, v2,
               W3, b3, Wnu, bnu, WB, bB, WC, bC, WD, bD):
    f32 = np.float32
    s1 = (g1 / np.sqrt(v1 + EPS)).astype(f32)
    t1 = ((b1 - m1) * s1 + be1).astype(f32)
    s2 = (g2 / np.sqrt(v2 + EPS)).astype(f32)
    t2 = ((b2 - m2) * s2 + be2).astype(f32)

    # mm3: lhsT = [WnuF | W3] -> psum rows 0:64 nu-pre, 64:128 hz-pre
    # nu = hz @ Wnu + bnu with hz = h2 @ W3 + b3, so fold through W3:
    # nu = h2 @ (W3 @ Wnu) + (b3 @ Wnu + bnu)
    WnuF = (np.asarray(W3, f32) @ np.asarray(Wnu, f32)).astype(f32)
    bnuF = (np.asarray(b3, f32) @ np.asarray(Wnu, f32) + np.asarray(bnu, f32)).astype(f32)
    W3nu = np.concatenate([WnuF, np.asarray(W3, f32)], axis=1)  # [128, 128]

    # G weights: row (u*64 + k); cols [n (64) | y (50)]
    WBr = np.asarray(WB, f32).reshape(LATENT, LATENT, UDIM)   # [k, n, u]
    WDr = np.asarray(WD, f32).reshape(LATENT, NY, UDIM)       # [k, y, u]
    WBDp = np.zeros((UDIM * LATENT, LATENT + NY), f32)
    WBDp[:, :LATENT] = WBr.transpose(2, 0, 1).reshape(UDIM * LATENT, LATENT)
    WBDp[:, LATENT:] = WDr.transpose(2, 0, 1).reshape(UDIM * LATENT, NY)
    # bias rows contracted with udt: [16, 114]
    bBr = np.asarray(bB, f32).reshape(LATENT, UDIM)           # [n, u]
    bDr = np.asarray(bD, f32).reshape(NY, UDIM)               # [y, u]
    WBDb = np.concatenate([bBr.T, bDr.T], axis=1)             # [16, 114]

    # G2 weights: row (n*64 + k), col y
    WCr = np.asarray(WC, f32).reshape(LATENT, NY, LATENT)     # [k, y, n]
    WCp = WCr.transpose(2, 0, 1).reshape(LATENT * LATENT, NY) # [(n,k), y]
    bCr = np.asarray(bC, f32).reshape(NY, LATENT).T           # [n, y]

    # selector matrices
    selZ = np.zeros((LATENT, NKC * 128), f32)
    for c in range(NKC):
        for m in range(128):
            selZ[2 * c + m // 64, 128 * c + m] = 1.0
    # lhsT chunk layout for wide weights: [128, nchunk*cols]
    WBDp_s = np.concatenate(
        [WBDp[128 * c:128 * (c + 1), :] for c in range(8)], axis=1)   # [128, 8*114]
    WCp_s = np.concatenate(
        [WCp[128 * c:128 * (c + 1), :] for c in range(32)], axis=1)   # [128, 32*50]

    # transposed streaming inputs
    zT0 = np.ascontiguousarray(np.asarray(zt, f32).T)                  # [64, B]
    Ut = np.asarray(U, f32)
    dtv = np.asarray(dt, f32)                                          # [B, 1]
    UX = np.concatenate(
        [Ut.transpose(0, 2, 1),                                        # [T, 16, B]
         np.broadcast_to(dtv.T[None], (T, 1, B))], axis=1)             # [T, 17, B]
    UX = np.ascontiguousarray(UX, f32)
    UDT = np.ascontiguousarray((Ut * dtv[None]).transpose(0, 2, 1), f32)  # [T, 16, B]

    import ml_dtypes
    bf16 = ml_dtypes.bfloat16
    per_core_const = {
        "W1": np.ascontiguousarray(np.asarray(W1, f32)),
        "W2": np.ascontiguousarray(np.asarray(W2, f32)),
        "W3nu": np.ascontiguousarray(W3nu),
        "s1": s1.reshape(H1, 1), "t1": t1.reshape(H1, 1),
        "s2": s2.reshape(H2, 1), "t2": t2.reshape(H2, 1),
        "b3": np.asarray(b3, f32).reshape(LATENT, 1),
        "bnuF": bnuF.reshape(LATENT, 1),
        "WBDp": np.ascontiguousarray(WBDp_s),
        "WBDb": np.ascontiguousarray(WBDb),
        "WCp": np.ascontiguousarray(WCp_s.astype(bf16)),
        "bCr": np.ascontiguousarray(bCr.astype(bf16)),
        "selZ": selZ.astype(bf16),
    }
    return per_core_const, zT0, UX, UDT


def _build_program():
    from concourse import bacc, mybir, tile
    from concourse import bass as cbass

    f32 = mybir.dt.float32
    bf16 = mybir.dt.bfloat16
    mmdt = getattr(mybir.dt, MM_DT)
    Act = mybir.ActivationFunctionType
    Alu = mybir.AluOpType

    nc = bacc.Bacc("TRN2", target_bir_lowering=False, debug=False, num_devices=1)

    def din(name, shape, dt=None):
        return nc.dram_tensor(name, list(shape), dt or mmdt, kind="ExternalInput").ap()

    zT0 = din("zT0", [LATENT, BS])
    UX = din("UX", [T, DIN - LATENT, BS])
    UDT = din("UDT", [T, UDIM, BS])
    W1 = din("W1", [DIN, H1])
    W2 = din("W2", [H1, H2])
    W3nu = din("W3nu", [H2, 128])
    s1 = din("s1", [H1, 1], f32); t1 = din("t1", [H1, 1], f32)
    s2 = din("s2", [H2, 1], f32); t2 = din("t2", [H2, 1], f32)
    b3 = din("b3", [LATENT, 1], f32); bnuF = din("bnuF", [LATENT, 1], f32)
    WBDp = din("WBDp", [128, 8 * (LATENT + NY)])
    WBDb = din("WBDb", [UDIM, LATENT + NY])
    WCp = din("WCp", [128, 32 * NY], bf16)
    bCr = din("bCr", [LATENT, NY], bf16)
    selZ = din("selZ", [LATENT, NKC * 128], bf16)

    ZT = nc.dram_tensor("ZT", [T, LATENT, BS], mmdt, kind="ExternalOutput").ap()
    YT = nc.dram_tensor("YT", [T, NY, BS], f32, kind="ExternalOutput").ap()

    NW = LATENT + NY  # 114

    with tile.TileContext(nc) as tc:
        with tc.tile_pool(name="const", bufs=1) as cpool, \
             tc.tile_pool(name="x", bufs=3) as xpool, \
             tc.tile_pool(name="h", bufs=3) as hpool, \
             tc.tile_pool(name="hz2", bufs=2) as hz2pool, \
             tc.tile_pool(name="small", bufs=4) as spool, \
             tc.tile_pool(name="g", bufs=3) as gpool, \
             tc.tile_pool(name="y", bufs=3) as ypool, \
             tc.tile_pool(name="udt", bufs=3) as upool, \
             tc.tile_pool(name="psmlp", bufs=2, space="PSUM") as psmlp, \
             tc.tile_pool(name="psbc", bufs=2, space="PSUM") as psbc, \
             tc.tile_pool(name="psA", bufs=1, space="PSUM") as psApool, \
             tc.tile_pool(name="psB", bufs=1, space="PSUM") as psBpool:

            def load_const(src, shape, tag, dt=None):
                t = cpool.tile(list(shape), dt or mmdt, tag=tag)
                nc.sync.dma_start(out=t[:, :], in_=src)
                return t

            W1s = load_const(W1, [DIN, H1], "cW1")
            W2s = load_const(W2, [H1, H2], "cW2")
            W3nus = load_const(W3nu, [H2, 128], "cW3nu")
            s1s = load_const(s1, [H1, 1], "cs1", f32); t1s = load_const(t1, [H1, 1], "ct1", f32)
            s2s = load_const(s2, [H2, 1], "cs2", f32); t2s = load_const(t2, [H2, 1], "ct2", f32)
            b3s = load_const(b3, [LATENT, 1], "cb3", f32)
            bnus = load_const(bnuF, [LATENT, 1], "cbnu", f32)
            WBDps = load_const(WBDp, [128, 8 * NW], "cWBDp")
            WBDbs = load_const(WBDb, [UDIM, NW], "cWBDb")
            WCps = load_const(WCp, [128, 32 * NY], "cWCp", bf16)
            bCrs = load_const(bCr, [LATENT, NY], "cbCr", bf16)
            selZs = load_const(selZ, [LATENT, NKC * 128], "cselZ", bf16)

            def mmr(ap):
                return ap

            # initial x tile: [z; u; dt]
            x_prev = xpool.tile([DIN, BS], mmdt)
            nc.sync.dma_start(out=x_prev[0:LATENT, :], in_=zT0)
            nc.sync.dma_start(out=x_prev[LATENT:DIN, :], in_=UX[0])

            for t in range(T):
                # ---- MLP ----
                ps1 = psmlp.tile([H1, BS], f32, tag="ps")
                nc.tensor.matmul(ps1[:, :], mmr(W1s[:, :]), mmr(x_prev[:, :]),
                                 start=True, stop=True)
                h1 = hpool.tile([H1, BS], mmdt, tag="h")
                nc.scalar.activation(h1[:, :], ps1[:, :], Act.Relu,
                                     bias=t1s[:, :], scale=s1s[:, :])

                ps2 = psmlp.tile([H2, BS], f32, tag="ps")
                nc.tensor.matmul(ps2[:, :], mmr(W2s[:, :]), mmr(h1[:, :]),
                                 start=True, stop=True)
                h2 = hpool.tile([H2, BS], mmdt, tag="h")
                nc.scalar.activation(h2[:, :], ps2[:, :], Act.Relu,
                                     bias=t2s[:, :], scale=s2s[:, :])

                ps3 = psmlp.tile([128, BS], f32, tag="ps")
                nc.tensor.matmul(ps3[:, :], mmr(W3nus[:, :]), mmr(h2[:, :]),
                                 start=True, stop=True)

                # HZ2 = [hz; hz]
                HZ2 = hz2pool.tile([128, BS], mmdt)
                nc.scalar.activation(HZ2[0:LATENT, :], ps3[LATENT:128, :],
                                     Act.Identity, bias=b3s[:, :])
                nc.sync.dma_start(out=HZ2[LATENT:128, :], in_=HZ2[0:LATENT, :])

                # Lam = exp(-exp(min(nu + bnuF, 2)))
                nuc = spool.tile([LATENT, BS], f32, tag="nu")
                nc.vector.tensor_scalar(nuc[:, :], ps3[0:LATENT, :],
                                        bnus[:, :], NU_MAX, Alu.add, Alu.min)
                ex = spool.tile([LATENT, BS], f32, tag="ex")
                nc.scalar.activation(ex[:, :], nuc[:, :], Act.Exp)
                Lam = spool.tile([LATENT, BS], f32, tag="lam")
                nc.scalar.activation(Lam[:, :], ex[:, :], Act.Exp, scale=-1.0)

                # ---- G path: psA = WBD.T @ (udt (x) hz) + bias rows ----
                udt_t = upool.tile([UDIM, BS], mmdt, tag="udt")
                nc.sync.dma_start(out=udt_t[:, :], in_=UDT[t])
                # UdtB: chunk c cols [256c:256c+256]; rows 0:64=udt[2c], 64:128=udt[2c+1]
                ub = upool.tile([128, NUC * BS], mmdt, tag="ub")
                nc.sync.dma_start(
                    out=ub[0:64, :].rearrange("p (c f) -> p c f", c=NUC),
                    in_=UDT[t, 0:UDIM:2, :].unsqueeze(0).broadcast_to([64, NUC, BS]))
                nc.sync.dma_start(
                    out=ub[64:128, :].rearrange("p (c f) -> p c f", c=NUC),
                    in_=UDT[t, 1:UDIM:2, :].unsqueeze(0).broadcast_to([64, NUC, BS]))
                gu = gpool.tile([128, NUC * BS], mmdt, tag="gu")
                nc.gpsimd.tensor_tensor(
                    gu[:, :].rearrange("p (c f) -> p c f", c=NUC),
                    ub[:, :].rearrange("p (c f) -> p c f", c=NUC),
                    HZ2[:, :].unsqueeze(1).broadcast_to([128, NUC, BS]),
                    Alu.mult)
                psA = psApool.tile([NW, BS], f32)
                for c in range(NUC):
                    nc.tensor.matmul(psA[:, :],
                                     mmr(WBDps[:, NW * c:NW * (c + 1)]),
                                     mmr(gu[:, BS * c:BS * (c + 1)]),
                                     start=(c == 0), stop=False)
                nc.tensor.matmul(psA[:, :], mmr(WBDbs[:, :]), mmr(udt_t[:, :]),
                                 start=False, stop=True)

                # ---- z1 = Lam * z + psA[0:64] ----
                zl = spool.tile([LATENT, BS], f32, tag="zl")
                nc.gpsimd.tensor_tensor(zl[:, :], Lam[:, :], x_prev[0:LATENT, :],
                                        Alu.mult)
                x_next = xpool.tile([DIN, BS], mmdt)
                nc.vector.tensor_tensor(x_next[0:LATENT, :], zl[:, :],
                                        psA[0:LATENT, :], Alu.add)
                zbf = spool.tile([LATENT, BS], bf16, tag="zbf")
                nc.scalar.activation(zbf[:, :], x_next[0:LATENT, :], Act.Identity,
                                     bias=0.0)
                nc.sync.dma_start(out=ZT[t], in_=x_next[0:LATENT, :])
                if t + 1 < T:
                    nc.sync.dma_start(out=x_next[LATENT:DIN, :], in_=UX[t + 1])

                # ---- G2 path: psB = WCp.T @ (z1 (x) hz) + bCr.T @ z1 ----
                psB = psBpool.tile([NY, BS], f32)
                GRP = 4
                for grp in range(NKC // GRP):
                    mega = psbc.tile([128, GRP * BS], f32, tag="bc")
                    for j in range(GRP):
                        c = GRP * grp + j
                        nc.tensor.matmul(mega[:, BS * j:BS * (j + 1)],
                                         selZs[:, 128 * c:128 * (c + 1)],
                                         zbf[:, :],
                                         start=True, stop=True)
                    g2 = gpool.tile([128, GRP * BS], bf16, tag="g2")
                    nc.vector.tensor_tensor(
                        g2[:, :].rearrange("p (c f) -> p c f", c=GRP),
                        mega[:, :].rearrange("p (c f) -> p c f", c=GRP),
                        HZ2[:, :].unsqueeze(1).broadcast_to([128, GRP, BS]),
                        Alu.mult)
                    for j in range(GRP):
                        c = GRP * grp + j
                        nc.tensor.matmul(psB[:, :],
                                         mmr(WCps[:, NY * c:NY * (c + 1)]),
                                         mmr(g2[:, BS * j:BS * (j + 1)]),
                                         start=(c == 0), stop=False)
                nc.tensor.matmul(psB[:, :], bCrs[:, :],
                                 zbf[:, :], start=False, stop=True)

                # ---- y = psA[64:114] + psB ----
                yb = ypool.tile([NY, BS], f32, tag="yb")
                nc.scalar.activation(yb[:, :], psB[:, :], Act.Identity, bias=0.0)
                yd = ypool.tile([NY, BS], f32, tag="yd")
                nc.scalar.activation(yd[:, :], psA[LATENT:NW, :], Act.Identity,
                                     bias=0.0)
                yt = ypool.tile([NY, BS], f32, tag="yt")
                nc.gpsimd.tensor_tensor(yt[:, :], yb[:, :], yd[:, :], Alu.add)
                nc.sync.dma_start(out=YT[t], in_=yt[:, :])

                x_prev = x_next

    nc.compile()
    return nc


_NC_CACHE = None
LAST_RESULT = None


def kernel(**inputs):
    global _NC_CACHE, LAST_RESULT
    import os
    from concourse.bass_utils import run_bass_kernel_spmd

    const, zT0, UX, UDT = _host_prep(**inputs)

    if _NC_CACHE is None:
        _NC_CACHE = _build_program()
    nc = _NC_CACHE

    in_maps = []
    for i in range(NCORES):
        sl = slice(i * BS, (i + 1) * BS)
        m = dict(const)
        m["zT0"] = np.ascontiguousarray(zT0[:, sl])
        m["UX"] = np.ascontiguousarray(UX[:, :, sl])
        m["UDT"] = np.ascontiguousarray(UDT[:, :, sl])
        in_maps.append(m)

    res = run_bass_kernel_spmd(nc, in_maps, core_ids=list(range(NCORES)),
                               trace=bool(os.environ.get("KERNEL_TRACE")))
    LAST_RESULT = res

    Z = np.empty((T, B, LATENT), np.float32)
    Y = np.empty((T, B, NY), np.float32)
    for i in range(NCORES):
        sl = slice(i * BS, (i + 1) * BS)
        Z[:, sl, :] = res.results[i]["ZT"].transpose(0, 2, 1)
        Y[:, sl, :] = res.results[i]["YT"].transpose(0, 2, 1)
    return Z, Y
